# revision 4
# baseline (speedup 1.0000x reference)
import numpy as np
import jax, jax.numpy as jnp
from functools import partial

B, P, V = 8, 16, 32
N_TH1 = 20
NPC, NNS, NOUT = 16384, 8192, 8192


def make_directions(n_th1):
    n_th2 = 2 * (n_th1 - 1) + 1
    th1 = np.linspace(-np.pi / 2, np.pi / 2, n_th1)
    th2 = np.linspace(-np.pi, np.pi, n_th2)
    c1, s1 = np.cos(th1), np.sin(th1)
    c2, s2 = np.cos(th2), np.sin(th2)
    ds = []
    for i in range(1, n_th1 - 1):
        for j in range(n_th2 - 1):
            ds.append([c1[i] * c2[j], c1[i] * s2[j], s1[i]])
    ds.append([c1[0] * c2[0], c1[0] * s2[0], s1[0]])
    ds.append([c1[-1] * c2[0], c1[-1] * s2[0], s1[-1]])
    return np.asarray(ds, np.float32)

DIRS_NP = make_directions(N_TH1)
D = DIRS_NP.shape[0]  # 686


def _spt(dirs, verts, smooth, trans):
    # per-batch-element: dirs [P,N,3], verts [P,V,3], smooth [P], trans [P,1,3]
    z = jnp.einsum('pvc,pnc->pvn', verts, dirs)
    zm = jnp.maximum(z, 0.0)
    pe = smooth[:, None, None]
    maxz = jnp.clip(jnp.max(z, axis=1, keepdims=True), 1e-30, 1e30)
    k = 1.0 / maxz
    zmk = jnp.clip(zm * k, 1e-30, 1e30)
    sum_zm_p = jnp.sum(zmk ** pe, axis=1, keepdims=True)
    h = sum_zm_p ** (1.0 / pe)
    s = jnp.clip(h, 1e-30, 1e30)
    dhdz = jnp.clip((zmk / s) ** (pe - 1.0), 1e-30, 1e30)
    surf = jnp.einsum('pvn,pvc->pnc', dhdz, verts) + trans
    h_out = jnp.clip(h / k, -1e30, 1e30).transpose(0, 2, 1)  # [P,N,1]
    return h_out, surf


def _distance(verts, smooth, trans, pts, normal_filter=None):
    local = pts - trans
    nrm = jnp.sqrt(jnp.clip(jnp.sum(local * local, -1, keepdims=True), 1e-40, 1e40))
    d = local / nrm
    h, surf = _spt(d, verts, smooth, trans)
    dist = nrm - h
    if normal_filter is not None:
        dist = jnp.where(normal_filter, 100.0, dist)
    return dist, surf, d


def _overlap(verts, smooth, trans):
    t = trans[:, 0, :]                              # [P,3]
    diff = t[None, :, :] - t[:, None, :]            # [i,j,3]
    dn = jnp.sqrt(jnp.clip(jnp.sum(diff * diff, -1, keepdims=True), 1e-20, 1e20))
    eyem = jnp.eye(P, dtype=t.dtype)[:, :, None]
    safe = jnp.array([1.0, 0.0, 0.0], dtype=t.dtype)
    d = diff / dn * (1.0 - eyem) + eyem * safe
    h, _ = _spt(d, verts, smooth, trans)
    sep = dn - h - jnp.swapaxes(h, 0, 1)
    return jax.nn.relu(-sep) * (1.0 - eyem)


def np_overlap(vertices, smoothness):
    # host-side float64 overlap: vertices [B,P,V,3], smoothness [B,P]
    v64 = vertices.astype(np.float64)
    trans = v64.mean(axis=2, keepdims=True)              # [B,P,1,3]
    verts = v64 - trans
    t = trans[:, :, 0, :]
    diff = t[:, None, :, :] - t[:, :, None, :]           # [B,i,j,3]
    dn = np.sqrt(np.clip((diff * diff).sum(-1, keepdims=True), 1e-20, 1e20))
    eyem = np.eye(P)[None, :, :, None]
    safe = np.array([1.0, 0.0, 0.0])
    d = diff / dn * (1.0 - eyem) + eyem * safe
    z = np.einsum('bpvc,bpnc->bpvn', verts, d)
    zm = np.maximum(z, 0.0)
    pe = smoothness.astype(np.float64)[:, :, None, None]
    maxz = np.clip(z.max(axis=2, keepdims=True), 1e-30, 1e30)
    k = 1.0 / maxz
    zmk = np.clip(zm * k, 1e-30, 1e30)
    h = (zmk ** pe).sum(axis=2, keepdims=True) ** (1.0 / pe)
    h_out = np.clip(h / k, -1e30, 1e30).transpose(0, 1, 3, 2)  # [B,i,j,1]
    sep = dn - h_out - np.swapaxes(h_out, 1, 2)
    ov = np.maximum(-sep, 0.0) * (1.0 - eyem)
    return ov.astype(np.float32)


def _forward_one(vertices, smoothness, pointcloud, nearsurf, out, dirs):
    # single batch element: vertices [P,V,3], smoothness [P], pointcloud [NPC,3]
    mean_v = jnp.mean(vertices, axis=1, keepdims=True)   # [P,1,3]
    local_v = vertices - mean_v
    x = jnp.broadcast_to(dirs[None], (P, D, 3))
    h, _ = _spt(x, local_v, smoothness, mean_v)          # [P,D,1]
    pc = jnp.broadcast_to(pointcloud[None], (P, NPC, 3))
    distance, _, _ = _distance(local_v, smoothness, mean_v, pc)
    scale = jnp.clip(jnp.max(h, axis=1, keepdims=True), 1e-10, 10.0)
    _, surf_pts, normals = _distance(local_v, smoothness, mean_v, x * scale + mean_v)
    surf_points = surf_pts.reshape(P * D, 3)
    sp2 = jnp.broadcast_to(surf_points[None], (P, P * D, 3))
    local_trans = jnp.transpose(mean_v, (1, 0, 2)) - mean_v          # [P,P,3]
    nf = jnp.einsum('ijc,idc->ijd', local_trans, normals)            # [P,P,D]
    normal_filter = nf.reshape(P, P * D)[..., None] < 0
    surf_distance, _, _ = _distance(local_v, smoothness, mean_v, sp2,
                                    normal_filter=normal_filter)
    sd = surf_distance.reshape(P, P, D, 1)
    df = 100.0 * jnp.eye(P, dtype=sd.dtype)[:, :, None, None]
    sd = jnp.where(df > 0, df, sd)
    surf_distance = sd.reshape(P, P * D, 1)
    ns = jnp.broadcast_to(nearsurf[None], (P, NNS, 3))
    nearsurf_dist, _, _ = _distance(local_v, smoothness, mean_v, ns)
    op = jnp.broadcast_to(out[None], (P, NOUT, 3))
    out_dist, _, _ = _distance(local_v, smoothness, mean_v, op)
    return distance, x, surf_distance, surf_points, nearsurf_dist, out_dist


_fwd_pmapped = None


def _get_fwd():
    global _fwd_pmapped
    if _fwd_pmapped is None:
        dirs = jnp.asarray(DIRS_NP)
        _fwd_pmapped = jax.pmap(partial(_forward_one, dirs=dirs),
                                devices=jax.devices()[:8])
    return _fwd_pmapped


def kernel(vertices, smoothness, pointcloud, nearsurf, out):
    vertices = np.asarray(vertices)
    smoothness = np.asarray(smoothness)
    fwd = _get_fwd()
    res = fwd(jnp.asarray(vertices), jnp.asarray(smoothness),
              jnp.asarray(pointcloud), jnp.asarray(nearsurf), jnp.asarray(out))
    overlap = np_overlap(vertices, smoothness)
    distance, x, surf_distance, surf_points, nearsurf_dist, out_dist = (
        np.asarray(r) for r in res)
    return (overlap, distance, x, surf_distance, surf_points,
            nearsurf_dist, out_dist)
